# revision 6
# baseline (speedup 1.0000x reference)
"""Trainium2 Bass kernel for prototypical-network kNN retrieval.

Computes, for S_SENT=256, L=128, D=768, C=21 classes:
  proto   = segment-mean of masked support embeddings per class   [C, D]
  logits  = -||q - proto_c||^2 expanded as 2 q.p - ||q||^2 - ||p||^2  [Nq, C]
  pred    = argmax_c logits                                       [Nq]

Strategy: data-parallel over tokens across 8 NeuronCores. Each core
computes partial per-class sums/counts from its support shard (one-hot
GEMM with emb blocks stationary so the accumulator lands directly in
proto^T layout), AllReduces the [C, D+1] partials, then computes
distances for its local query shard (PE transposes Q tiles, cross-term
matmuls with tiny moving operands, fused DVE combine + argmax).
"""

import sys

if "/opt/trn_rl_repo" not in sys.path:
    sys.path.insert(0, "/opt/trn_rl_repo")

import numpy as np

import concourse.bacc as bacc
import concourse.mybir as mybir
import concourse.tile as tile
import concourse.masks as masks
from concourse.bass_utils import run_bass_kernel_spmd

N_CORES = 8
S_SENT, L, D = 256, 128, 768
C = 21
NTOK = S_SENT * L            # 32768 total tokens (support == query count)
TOK = NTOK // N_CORES        # 4096 tokens per core
NT = TOK // 128              # 32 tiles of 128 tokens
GT = 4                       # tiles per DMA group
NG = NT // GT                # 8 groups
ND = D // 128                # 6 d-chunks of 128
CP = 32                      # padded per-chunk column stride in PSUM accum

f32 = mybir.dt.float32
i32 = mybir.dt.int32
u32 = mybir.dt.uint32
Alu = mybir.AluOpType


def build_kernel():
    nc = bacc.Bacc(
        "TRN2", target_bir_lowering=False, debug=False, num_devices=N_CORES
    )

    s_emb = nc.dram_tensor("support_emb", [TOK, D], f32, kind="ExternalInput")
    q_emb = nc.dram_tensor("query_emb", [TOK, D], f32, kind="ExternalInput")
    s_lab = nc.dram_tensor("support_label", [TOK], i32, kind="ExternalInput")
    s_msk = nc.dram_tensor("support_text_mask", [TOK], i32, kind="ExternalInput")
    logits_out = nc.dram_tensor("logits", [TOK, C], f32, kind="ExternalOutput")
    pred_out = nc.dram_tensor("pred", [TOK], i32, kind="ExternalOutput")

    with tile.TileContext(nc) as tc:
        with (
            tc.tile_pool(name="const", bufs=1) as constp,
            tc.tile_pool(name="io", bufs=3) as iop,
            tc.tile_pool(name="qt", bufs=NT) as qtp,
            tc.tile_pool(name="small", bufs=2) as smallp,
            tc.tile_pool(name="ps_acc", bufs=1, space="PSUM") as ps_acc,
            tc.tile_pool(name="ps_t", bufs=2, space="PSUM") as ps_t,
            tc.tile_pool(name="ps_misc", bufs=2, space="PSUM") as ps_misc,
            tc.tile_pool(name="dram", bufs=1, space="DRAM") as dramp,
        ):
            # ---------------- constants ----------------
            ident = constp.tile([128, 128], f32)
            masks.make_identity(nc, ident[:])

            iota_i = constp.tile([128, C], i32)
            nc.gpsimd.iota(iota_i[:], pattern=[[1, C]], base=0, channel_multiplier=0)
            iota_f = constp.tile([128, C], f32)
            nc.vector.tensor_copy(iota_f[:], iota_i[:])

            ones_col = constp.tile([128, 1], f32)
            nc.vector.memset(ones_col[:], 1.0)
            # labels/mask for the support shard: [128, NT] with partition =
            # token % 128, column = tile index (strided 4B gather, small).
            lab_i = constp.tile([128, NT], i32)
            nc.gpsimd.dma_start(lab_i[:], s_lab.ap().rearrange("(j p) -> p j", p=128))
            msk_i = constp.tile([128, NT], i32)
            nc.gpsimd.dma_start(msk_i[:], s_msk.ap().rearrange("(j p) -> p j", p=128))
            lab_f = constp.tile([128, NT], f32)
            nc.vector.tensor_copy(lab_f[:], lab_i[:])
            msk_f = constp.tile([128, NT], f32)
            nc.vector.tensor_copy(msk_f[:], msk_i[:])

            # ---------------- support pass ----------------
            # protoT_ps[:, c, 0:C] accumulates emb_chunk^T @ onehot = [d, C];
            # slot c=ND holds the counts row. All in ONE psum bank: exactly one
            # start=True (clears whole-bank has_written) and one stop=True.
            protoT_ps = ps_acc.tile([128, ND + 1, CP], f32, tag="protoT")

            s_view = s_emb.ap().rearrange("(g a p) d -> g p a d", p=128, a=GT)
            for g in range(NG):
                emb_g = iop.tile([128, GT, D], f32, tag="in_group")
                nc.sync.dma_start(emb_g[:], s_view[g])
                for a in range(GT):
                    j = g * GT + a
                    oh = smallp.tile([128, C], f32, tag="onehot")
                    # onehot = (iota == label) * mask
                    nc.vector.tensor_scalar(
                        oh[:],
                        iota_f[:],
                        lab_f[:, j : j + 1],
                        msk_f[:, j : j + 1],
                        op0=Alu.is_equal,
                        op1=Alu.mult,
                    )
                    def counts_mm():
                        nc.tensor.matmul(
                            protoT_ps[0:1, ND, 0:C],
                            lhsT=ones_col[:],
                            rhs=oh[:],
                            start=False,
                            stop=False,
                        )

                    # sim bookkeeping: start/stop must be on a matmul covering
                    # all 128 partitions, so counts goes last except on the
                    # final tile (where the stop-carrying chunk must be last).
                    if j == NT - 1:
                        counts_mm()
                    for c in range(ND):
                        nc.tensor.matmul(
                            protoT_ps[:, c, 0:C],
                            lhsT=emb_g[:, a, c * 128 : (c + 1) * 128],
                            rhs=oh[:],
                            start=(j == 0 and c == 0),
                            stop=(j == NT - 1 and c == ND - 1),
                        )
                    if j != NT - 1:
                        counts_mm()

            # ---------------- all-reduce partials ----------------
            partial = constp.tile([128, 148], f32, tag="partial")
            nc.vector.memset(partial[:], 0.0)
            nc.scalar.copy(
                partial[:, 0 : ND * C].rearrange("p (k c) -> p k c", c=C),
                protoT_ps[:, 0:ND, 0:C],
            )
            nc.vector.tensor_copy(
                partial[0:1, ND * C : ND * C + C], protoT_ps[0:1, ND, 0:C]
            )

            cc_in = dramp.tile([128, 148], f32)
            cc_out = dramp.tile([128, 148], f32)
            nc.sync.dma_start(cc_in[:], partial[:])
            nc.gpsimd.collective_compute(
                "AllReduce",
                Alu.add,
                replica_groups=[list(range(N_CORES))],
                ins=[cc_in.opt()],
                outs=[cc_out.opt()],
            )
            total = constp.tile([128, 148], f32, tag="total")
            nc.sync.dma_start(total[:], cc_out[:])

            # ---------------- query pass A: load, transpose, ||q||^2 ----------
            q_view = q_emb.ap().rearrange("(g a p) d -> g p a d", p=128, a=GT)
            q2all = constp.tile([128, NT], f32, tag="q2all")
            qts = []
            q_groups = []
            for g in range(NG):
                q_g = iop.tile([128, GT, D], f32, tag="in_group")
                nc.sync.dma_start(q_g[:], q_view[g])
                q_groups.append(q_g)
                for a in range(GT):
                    j = g * GT + a
                    qt = qtp.tile([128, D], f32, tag="qt")
                    qts.append(qt)
                    ps1 = ps_t.tile([128, 512], f32, tag="t1")
                    ps2 = ps_t.tile([128, 256], f32, tag="t2")
                    for c in range(4):
                        nc.tensor.matmul(
                            ps1[:, c * 128 : (c + 1) * 128],
                            lhsT=q_g[:, a, c * 128 : (c + 1) * 128],
                            rhs=ident[:],
                            is_transpose=True,
                            start=(c == 0),
                            stop=(c == 3),
                        )
                    for c in range(4, ND):
                        nc.tensor.matmul(
                            ps2[:, (c - 4) * 128 : (c - 3) * 128],
                            lhsT=q_g[:, a, c * 128 : (c + 1) * 128],
                            rhs=ident[:],
                            is_transpose=True,
                            start=(c == 4),
                            stop=(c == ND - 1),
                        )
                    nc.scalar.copy(qt[:, 0:512], ps1[:])
                    nc.scalar.copy(qt[:, 512:D], ps2[:])
                    # ||q||^2 for this tile -> q2all[:, j]
                    qsq = smallp.tile([128, D], f32, tag="qsq")
                    nc.vector.scalar_tensor_tensor(
                        out=qsq[:],
                        in0=q_g[:, a],
                        scalar=0.0,
                        in1=q_g[:, a],
                        op0=Alu.add,
                        op1=Alu.mult,
                        accum_out=q2all[:, j : j + 1],
                    )

            # ---------------- finalize prototypes ----------------
            counts_row = total[0:1, ND * C : ND * C + C]
            cnt1 = constp.tile([1, C], f32, tag="cnt1")
            nc.vector.tensor_scalar_max(cnt1[:], counts_row, 1.0)
            recip = constp.tile([1, C], f32, tag="recip")
            nc.vector.reciprocal(recip[:], cnt1[:])
            recip2 = constp.tile([1, C], f32, tag="recip2")
            nc.vector.tensor_scalar_mul(recip2[:], recip[:], 2.0)

            # broadcast 2/count to all 128 partitions (K=1 matmuls hang HW,
            # use the gpsimd partition-broadcast ucode op instead)
            rb_b = constp.tile([128, C], f32, tag="rbb")
            nc.gpsimd.partition_broadcast(rb_b[:], recip2[:])
            # protoT_s = (2/count) * class_sum^T   (= 2 * proto^T), [d, C] chunks
            protoT_s = constp.tile([128, ND, CP], f32, tag="protoTs")
            for c in range(ND):
                nc.vector.tensor_tensor(
                    protoT_s[:, c, 0:C],
                    total[:, c * C : (c + 1) * C],
                    rb_b[:],
                    op=Alu.mult,
                )

            # p2 = sum_d proto^2 = 0.25 * sum_d (2 proto)^2 ; build -p2 bcast
            sq = constp.tile([128, ND, CP], f32, tag="sq")
            nc.scalar.square(sq[:, :, 0:C], protoT_s[:, :, 0:C])
            p2_ps = ps_misc.tile([1, C], f32, tag="misc")
            for c in range(ND):
                nc.tensor.matmul(
                    p2_ps[:],
                    lhsT=ones_col[:],
                    rhs=sq[:, c, 0:C],
                    start=(c == 0),
                    stop=(c == ND - 1),
                )
            negp2_row = constp.tile([1, C], f32, tag="negp2row")
            nc.scalar.mul(negp2_row[:], p2_ps[:], -0.25)
            negp2_b = constp.tile([128, C], f32, tag="negp2b")
            nc.gpsimd.partition_broadcast(negp2_b[:], negp2_row[:])

            # ---------------- query pass B: cross terms + argmax ------------
            pred_f = constp.tile([128, NT], f32, tag="predf")
            lg_view = logits_out.ap().rearrange("(g a p) c -> g p a c", p=128, a=GT)
            for g in range(NG):
                lg_g = smallp.tile([128, GT, C], f32, tag="lgroup")
                for a in range(GT):
                    j = g * GT + a
                    qt = qts[j]
                    cr = ps_misc.tile([128, C], f32, tag="misc")
                    for c in range(ND):
                        nc.tensor.matmul(
                            cr[:],
                            lhsT=qt[:, c * 128 : (c + 1) * 128],
                            rhs=protoT_s[:, c, 0:C],
                            start=(c == 0),
                            stop=(c == ND - 1),
                        )
                    # logits = (cross - q2) + (-p2)
                    nc.vector.scalar_tensor_tensor(
                        out=lg_g[:, a],
                        in0=cr[:],
                        scalar=q2all[:, j : j + 1],
                        in1=negp2_b[:],
                        op0=Alu.subtract,
                        op1=Alu.add,
                    )
                    mx8 = smallp.tile([128, 8], f32, tag="mx8")
                    ix8 = smallp.tile([128, 8], u32, tag="ix8")
                    nc.vector.max(mx8[:], lg_g[:, a])
                    nc.vector.max_index(ix8[:], mx8[:], lg_g[:, a])
                    nc.vector.tensor_copy(pred_f[:, j : j + 1], ix8[:, 0:1])
                nc.gpsimd.dma_start(lg_view[g], lg_g[:])

            # ---------------- pred: transpose + store ----------------
            pt_ps = ps_misc.tile([NT, 128], f32, tag="misc")
            nc.tensor.transpose(pt_ps[:], pred_f[:], ident[:])
            pred_i = constp.tile([NT, 128], i32, tag="predi")
            nc.vector.tensor_copy(pred_i[:], pt_ps[:])
            nc.gpsimd.dma_start(
                pred_out.ap().rearrange("(j p) -> j p", p=128), pred_i[:]
            )

    nc.compile()
    return nc


_CACHE = {}


def get_compiled():
    if "nc" not in _CACHE:
        _CACHE["nc"] = build_kernel()
    return _CACHE["nc"]


def make_in_maps(support_emb, query_emb, support_label, support_text_mask):
    se = np.ascontiguousarray(np.asarray(support_emb, np.float32).reshape(NTOK, D))
    qe = np.ascontiguousarray(np.asarray(query_emb, np.float32).reshape(NTOK, D))
    sl = np.ascontiguousarray(np.asarray(support_label, np.int32).reshape(NTOK))
    sm = np.ascontiguousarray(np.asarray(support_text_mask, np.int32).reshape(NTOK))
    in_maps = []
    for k in range(N_CORES):
        s = slice(k * TOK, (k + 1) * TOK)
        in_maps.append(
            {
                "support_emb": se[s],
                "query_emb": qe[s],
                "support_label": sl[s],
                "support_text_mask": sm[s],
            }
        )
    return in_maps


def kernel(support_emb, query_emb, support_label, support_text_mask):
    nc = get_compiled()
    in_maps = make_in_maps(support_emb, query_emb, support_label, support_text_mask)
    res = run_bass_kernel_spmd(nc, in_maps, core_ids=list(range(N_CORES)))
    logits = np.concatenate(
        [res.results[k]["logits"] for k in range(N_CORES)], axis=0
    )
    pred = np.concatenate([res.results[k]["pred"] for k in range(N_CORES)], axis=0)
    return logits, pred


if __name__ == "__main__":
    from reference import setup_inputs, reference

    inputs = {k: np.asarray(v) for k, v in setup_inputs().items()}
    exp_logits, exp_pred = [np.asarray(x) for x in reference(**inputs)]
    got_logits, got_pred = kernel(**inputs)
    le = np.abs(got_logits - exp_logits)
    rel = np.linalg.norm(got_logits - exp_logits) / np.linalg.norm(exp_logits)
    nm = int((got_pred != exp_pred).sum())
    print(f"logits maxabs {le.max():.4e} rel {rel:.4e} pred mismatches {nm}/{NTOK}")


# revision 7
# speedup vs baseline: 550.0272x; 550.0272x over previous
"""Trainium2 Bass kernel for prototypical-network kNN retrieval.

Computes, for S_SENT=256, L=128, D=768, C=21 classes:
  proto   = segment-mean of masked support embeddings per class   [C, D]
  logits  = -||q - proto_c||^2 expanded as 2 q.p - ||q||^2 - ||p||^2  [Nq, C]
  pred    = argmax_c logits                                       [Nq]

Strategy: data-parallel over tokens across 8 NeuronCores. Each core
computes partial per-class sums/counts from its support shard (one-hot
GEMM with emb blocks stationary so the accumulator lands directly in
proto^T layout), AllReduces the [C, D+1] partials, then computes
distances for its local query shard (PE transposes Q tiles, cross-term
matmuls with tiny moving operands, fused DVE combine + argmax).
"""

import sys

if "/opt/trn_rl_repo" not in sys.path:
    sys.path.insert(0, "/opt/trn_rl_repo")

import numpy as np

import concourse.bacc as bacc
import concourse.mybir as mybir
import concourse.tile as tile
import concourse.masks as masks
from concourse.bass_utils import run_bass_kernel_spmd

N_CORES = 8
S_SENT, L, D = 256, 128, 768
C = 21
NTOK = S_SENT * L            # 32768 total tokens (support == query count)
TOK = NTOK // N_CORES        # 4096 tokens per core
NT = TOK // 128              # 32 tiles of 128 tokens
GT = 4                       # tiles per DMA group
NG = NT // GT                # 8 groups
ND = D // 128                # 6 d-chunks of 128
CP = 32                      # padded per-chunk column stride in PSUM accum

f32 = mybir.dt.float32
i32 = mybir.dt.int32
u32 = mybir.dt.uint32
Alu = mybir.AluOpType


def build_kernel(reps: int = 1):
    nc = bacc.Bacc(
        "TRN2", target_bir_lowering=False, debug=False, num_devices=N_CORES
    )

    s_emb = nc.dram_tensor("support_emb", [TOK, D], f32, kind="ExternalInput")
    q_emb = nc.dram_tensor("query_emb", [TOK, D], f32, kind="ExternalInput")
    s_lab = nc.dram_tensor("support_label", [TOK], i32, kind="ExternalInput")
    s_msk = nc.dram_tensor("support_text_mask", [TOK], i32, kind="ExternalInput")
    logits_out = nc.dram_tensor("logits", [TOK, C], f32, kind="ExternalOutput")
    pred_out = nc.dram_tensor("pred", [TOK], i32, kind="ExternalOutput")

    with tile.TileContext(nc) as tc:
        with (
            tc.tile_pool(name="const", bufs=1) as constp,
            tc.tile_pool(name="io", bufs=3) as iop,
            tc.tile_pool(name="qt", bufs=NT) as qtp,
            tc.tile_pool(name="small", bufs=2) as smallp,
            tc.tile_pool(name="ps_acc", bufs=1, space="PSUM") as ps_acc,
            tc.tile_pool(name="ps_t", bufs=2, space="PSUM") as ps_t,
            tc.tile_pool(name="ps_misc", bufs=2, space="PSUM") as ps_misc,
            tc.tile_pool(name="dram", bufs=1, space="DRAM") as dramp,
        ):
            # ---------------- constants ----------------
            ident = constp.tile([128, 128], f32)
            masks.make_identity(nc, ident[:])

            iota_i = constp.tile([128, C], i32)
            nc.gpsimd.iota(iota_i[:], pattern=[[1, C]], base=0, channel_multiplier=0)
            iota_f = constp.tile([128, C], f32)
            nc.vector.tensor_copy(iota_f[:], iota_i[:])

            ones_col = constp.tile([128, 1], f32)
            nc.vector.memset(ones_col[:], 1.0)
            # labels/mask for the support shard: [128, NT] with partition =
            # token % 128, column = tile index (strided 4B gather, small).
            lab_i = constp.tile([128, NT], i32)
            nc.gpsimd.dma_start(lab_i[:], s_lab.ap().rearrange("(j p) -> p j", p=128))
            msk_i = constp.tile([128, NT], i32)
            nc.gpsimd.dma_start(msk_i[:], s_msk.ap().rearrange("(j p) -> p j", p=128))
            lab_f = constp.tile([128, NT], f32)
            nc.vector.tensor_copy(lab_f[:], lab_i[:])
            msk_f = constp.tile([128, NT], f32)
            nc.vector.tensor_copy(msk_f[:], msk_i[:])

            for _rep in range(reps):
                _emit_once(nc, tc, iop, qtp, smallp, ps_acc, ps_t, ps_misc, dramp,
                           s_emb, q_emb, logits_out, pred_out,
                           ident, iota_f, ones_col, lab_f, msk_f, constp, _rep)

    nc.compile()
    return nc


def _emit_once(nc, tc, iop, qtp, smallp, ps_acc, ps_t, ps_misc, dramp,
               s_emb, q_emb, logits_out, pred_out,
               ident, iota_f, ones_col, lab_f, msk_f, constp, _rep):
            # ---------------- support pass ----------------
            # protoT_ps[:, c, 0:C] accumulates emb_chunk^T @ onehot = [d, C];
            # slot c=ND holds the counts row. All in ONE psum bank: exactly one
            # start=True (clears whole-bank has_written) and one stop=True.
            protoT_ps = ps_acc.tile([128, ND + 1, CP], f32, tag="protoT")

            s_view = s_emb.ap().rearrange("(g a p) d -> g p a d", p=128, a=GT)
            for g in range(NG):
                emb_g = iop.tile([128, GT, D], f32, tag="in_group")
                nc.sync.dma_start(emb_g[:], s_view[g])
                for a in range(GT):
                    j = g * GT + a
                    oh = smallp.tile([128, C], f32, tag="onehot")
                    # onehot = (iota == label) * mask
                    nc.vector.tensor_scalar(
                        oh[:],
                        iota_f[:],
                        lab_f[:, j : j + 1],
                        msk_f[:, j : j + 1],
                        op0=Alu.is_equal,
                        op1=Alu.mult,
                    )
                    def counts_mm():
                        nc.tensor.matmul(
                            protoT_ps[0:1, ND, 0:C],
                            lhsT=ones_col[:],
                            rhs=oh[:],
                            start=False,
                            stop=False,
                        )

                    # sim bookkeeping: start/stop must be on a matmul covering
                    # all 128 partitions, so counts goes last except on the
                    # final tile (where the stop-carrying chunk must be last).
                    if j == NT - 1:
                        counts_mm()
                    for c in range(ND):
                        nc.tensor.matmul(
                            protoT_ps[:, c, 0:C],
                            lhsT=emb_g[:, a, c * 128 : (c + 1) * 128],
                            rhs=oh[:],
                            start=(j == 0 and c == 0),
                            stop=(j == NT - 1 and c == ND - 1),
                        )
                    if j != NT - 1:
                        counts_mm()

            # ---------------- all-reduce partials ----------------
            partial = constp.tile([128, 148], f32, tag="partial{}".format(_rep))
            nc.vector.memset(partial[:], 0.0)
            nc.scalar.copy(
                partial[:, 0 : ND * C].rearrange("p (k c) -> p k c", c=C),
                protoT_ps[:, 0:ND, 0:C],
            )
            nc.vector.tensor_copy(
                partial[0:1, ND * C : ND * C + C], protoT_ps[0:1, ND, 0:C]
            )

            cc_in = dramp.tile([128, 148], f32)
            cc_out = dramp.tile([128, 148], f32)
            nc.sync.dma_start(cc_in[:], partial[:])
            nc.gpsimd.collective_compute(
                "AllReduce",
                Alu.add,
                replica_groups=[list(range(N_CORES))],
                ins=[cc_in.opt()],
                outs=[cc_out.opt()],
            )
            total = constp.tile([128, 148], f32, tag="total{}".format(_rep))
            nc.sync.dma_start(total[:], cc_out[:])

            # ---------------- query pass A: load, transpose, ||q||^2 ----------
            q_view = q_emb.ap().rearrange("(g a p) d -> g p a d", p=128, a=GT)
            q2all = constp.tile([128, NT], f32, tag="q2all{}".format(_rep))
            qts = []
            q_groups = []
            for g in range(NG):
                q_g = iop.tile([128, GT, D], f32, tag="in_group")
                nc.sync.dma_start(q_g[:], q_view[g])
                q_groups.append(q_g)
                for a in range(GT):
                    j = g * GT + a
                    qt = qtp.tile([128, D], f32, tag="qt")
                    qts.append(qt)
                    ps1 = ps_t.tile([128, 512], f32, tag="t1")
                    ps2 = ps_t.tile([128, 256], f32, tag="t2")
                    for c in range(4):
                        nc.tensor.matmul(
                            ps1[:, c * 128 : (c + 1) * 128],
                            lhsT=q_g[:, a, c * 128 : (c + 1) * 128],
                            rhs=ident[:],
                            is_transpose=True,
                            start=(c == 0),
                            stop=(c == 3),
                        )
                    for c in range(4, ND):
                        nc.tensor.matmul(
                            ps2[:, (c - 4) * 128 : (c - 3) * 128],
                            lhsT=q_g[:, a, c * 128 : (c + 1) * 128],
                            rhs=ident[:],
                            is_transpose=True,
                            start=(c == 4),
                            stop=(c == ND - 1),
                        )
                    nc.scalar.copy(qt[:, 0:512], ps1[:])
                    nc.scalar.copy(qt[:, 512:D], ps2[:])
                    # ||q||^2 for this tile -> q2all[:, j]
                    qsq = smallp.tile([128, D], f32, tag="qsq")
                    nc.vector.scalar_tensor_tensor(
                        out=qsq[:],
                        in0=q_g[:, a],
                        scalar=0.0,
                        in1=q_g[:, a],
                        op0=Alu.add,
                        op1=Alu.mult,
                        accum_out=q2all[:, j : j + 1],
                    )

            # ---------------- finalize prototypes ----------------
            counts_row = total[0:1, ND * C : ND * C + C]
            cnt1 = constp.tile([1, C], f32, tag="cnt1{}".format(_rep))
            nc.vector.tensor_scalar_max(cnt1[:], counts_row, 1.0)
            recip = constp.tile([1, C], f32, tag="recip{}".format(_rep))
            nc.vector.reciprocal(recip[:], cnt1[:])
            recip2 = constp.tile([1, C], f32, tag="recip2{}".format(_rep))
            nc.vector.tensor_scalar_mul(recip2[:], recip[:], 2.0)

            # broadcast 2/count to all 128 partitions (K=1 matmuls hang HW,
            # use the gpsimd partition-broadcast ucode op instead)
            rb_b = constp.tile([128, C], f32, tag="rbb{}".format(_rep))
            nc.gpsimd.partition_broadcast(rb_b[:], recip2[:])
            # protoT_s = (2/count) * class_sum^T   (= 2 * proto^T), [d, C] chunks
            protoT_s = constp.tile([128, ND, CP], f32, tag="protoTs{}".format(_rep))
            for c in range(ND):
                nc.vector.tensor_tensor(
                    protoT_s[:, c, 0:C],
                    total[:, c * C : (c + 1) * C],
                    rb_b[:],
                    op=Alu.mult,
                )

            # p2 = sum_d proto^2 = 0.25 * sum_d (2 proto)^2 ; build -p2 bcast
            sq = constp.tile([128, ND, CP], f32, tag="sq{}".format(_rep))
            nc.scalar.square(sq[:, :, 0:C], protoT_s[:, :, 0:C])
            p2_ps = ps_misc.tile([1, C], f32, tag="misc")
            for c in range(ND):
                nc.tensor.matmul(
                    p2_ps[:],
                    lhsT=ones_col[:],
                    rhs=sq[:, c, 0:C],
                    start=(c == 0),
                    stop=(c == ND - 1),
                )
            negp2_row = constp.tile([1, C], f32, tag="negp2row{}".format(_rep))
            nc.scalar.mul(negp2_row[:], p2_ps[:], -0.25)
            negp2_b = constp.tile([128, C], f32, tag="negp2b{}".format(_rep))
            nc.gpsimd.partition_broadcast(negp2_b[:], negp2_row[:])

            # ---------------- query pass B: cross terms + argmax ------------
            pred_f = constp.tile([128, NT], f32, tag="predf{}".format(_rep))
            lg_view = logits_out.ap().rearrange("(g a p) c -> g p a c", p=128, a=GT)
            for g in range(NG):
                lg_g = smallp.tile([128, GT, C], f32, tag="lgroup")
                for a in range(GT):
                    j = g * GT + a
                    qt = qts[j]
                    cr = ps_misc.tile([128, C], f32, tag="misc")
                    for c in range(ND):
                        nc.tensor.matmul(
                            cr[:],
                            lhsT=qt[:, c * 128 : (c + 1) * 128],
                            rhs=protoT_s[:, c, 0:C],
                            start=(c == 0),
                            stop=(c == ND - 1),
                        )
                    # logits = (cross - q2) + (-p2)
                    nc.vector.scalar_tensor_tensor(
                        out=lg_g[:, a],
                        in0=cr[:],
                        scalar=q2all[:, j : j + 1],
                        in1=negp2_b[:],
                        op0=Alu.subtract,
                        op1=Alu.add,
                    )
                    mx8 = smallp.tile([128, 8], f32, tag="mx8")
                    ix8 = smallp.tile([128, 8], u32, tag="ix8")
                    nc.vector.max(mx8[:], lg_g[:, a])
                    nc.vector.max_index(ix8[:], mx8[:], lg_g[:, a])
                    nc.vector.tensor_copy(pred_f[:, j : j + 1], ix8[:, 0:1])
                nc.gpsimd.dma_start(lg_view[g], lg_g[:])

            # ---------------- pred: transpose + store ----------------
            pt_ps = ps_misc.tile([NT, 128], f32, tag="misc")
            nc.tensor.transpose(pt_ps[:], pred_f[:], ident[:])
            pred_i = constp.tile([NT, 128], i32, tag="predi{}".format(_rep))
            nc.vector.tensor_copy(pred_i[:], pt_ps[:])
            nc.gpsimd.dma_start(
                pred_out.ap().rearrange("(j p) -> j p", p=128), pred_i[:]
            )


_CACHE = {}


def get_compiled():
    if "nc" not in _CACHE:
        _CACHE["nc"] = build_kernel()
    return _CACHE["nc"]


def make_in_maps(support_emb, query_emb, support_label, support_text_mask):
    se = np.ascontiguousarray(np.asarray(support_emb, np.float32).reshape(NTOK, D))
    qe = np.ascontiguousarray(np.asarray(query_emb, np.float32).reshape(NTOK, D))
    sl = np.ascontiguousarray(np.asarray(support_label, np.int32).reshape(NTOK))
    sm = np.ascontiguousarray(np.asarray(support_text_mask, np.int32).reshape(NTOK))
    in_maps = []
    for k in range(N_CORES):
        s = slice(k * TOK, (k + 1) * TOK)
        in_maps.append(
            {
                "support_emb": se[s],
                "query_emb": qe[s],
                "support_label": sl[s],
                "support_text_mask": sm[s],
            }
        )
    return in_maps


def kernel(support_emb, query_emb, support_label, support_text_mask):
    nc = get_compiled()
    in_maps = make_in_maps(support_emb, query_emb, support_label, support_text_mask)
    res = run_bass_kernel_spmd(nc, in_maps, core_ids=list(range(N_CORES)))
    logits = np.concatenate(
        [res.results[k]["logits"] for k in range(N_CORES)], axis=0
    )
    pred = np.concatenate([res.results[k]["pred"] for k in range(N_CORES)], axis=0)
    return logits, pred


if __name__ == "__main__":
    from reference import setup_inputs, reference

    inputs = {k: np.asarray(v) for k, v in setup_inputs().items()}
    exp_logits, exp_pred = [np.asarray(x) for x in reference(**inputs)]
    got_logits, got_pred = kernel(**inputs)
    le = np.abs(got_logits - exp_logits)
    rel = np.linalg.norm(got_logits - exp_logits) / np.linalg.norm(exp_logits)
    nm = int((got_pred != exp_pred).sum())
    print(f"logits maxabs {le.max():.4e} rel {rel:.4e} pred mismatches {nm}/{NTOK}")
